# revision 1
# baseline (speedup 1.0000x reference)
"""Trainium2 Bass kernel for nn_AggregationLayer (pooling).

Computation (reference):
    fact: [N,H], elements_p: [M,H], weights: [H,H]   (N=32768, M=4096, H=768)
    fact_Q = max(fact, axis=0)                     # [1,H]
    f2e = softmax(fact_Q @ W @ ep.T)               # [1,M]
    ep_ = f2e @ ep                                 # [1,H]
    e2f = softmax_n(fact @ W @ ep_.T)              # [N,1]
    fact_ = e2f.T @ fact                           # [1,H]
    returns (fact_, ep_)

Distribution over 8 NeuronCores:
    fact sharded along N (4096 rows per core, sent in 2 layouts:
    float32r transposed [H, N/8] DMA'd straight into the resident PE
    matvec tile — f32r storage is plain fp32 bytes, giving full-rate
    PE matmuls while the col-max reduce stays fp32-exact — and bf16
    row-major for the final weighted row-sum), elements_p sharded
    along M (512 rows per core, both layouts f32r), weights replicated
    f32r (W and W.T). Cross-core: one AllReduce(max) for fact_Q and
    one AllGather of a [128, 8] column-form payload (accep transposed
    to h-columns + the two softmax stats), combined on every core with
    slot-order-invariant math. A remote_dma_broadcast SBUF-to-SBUF
    all-gather path exists behind COMM="rdma" (correct on HW, but the
    per-broadcast Q7 SWDGE descriptor generation costs more per rep
    than ncfw collectives). The final e2f softmax partials (one scalar
    max, one scalar sum, one [768] weighted sum per core) are combined
    on host.
"""
import sys

sys.path.insert(0, "/opt/trn_rl_repo")

import numpy as np
import ml_dtypes

import concourse.bass as bass
import concourse.tile as tile
import concourse.mybir as mybir
from concourse import bacc, bass_utils

N, M, H = 32768, 4096, 768
NCORES = 8
NS = N // NCORES          # 4096 fact rows per core
MS = M // NCORES          # 512 ep rows per core
HC = H // 128             # 6 h-chunks of 128
NB = NS // 512            # 8 psum banks for e2f
NT = NS // 128            # 32 acc tiles
MQ = MS // 128            # 4 accep tiles

F32 = mybir.dt.float32
F32R = mybir.dt.float32r
BF16 = mybir.dt.bfloat16
RG = [list(range(NCORES))]

# PE clock-keeper matmul counts: W1 groups during the factT load,
# W2 during the AG1 (col-max exchange) flight, W3 during AG2.
W1, W2, W3 = 3, 25, 18

# cross-core exchange implementation: "rdma" = remote_dma_broadcast
# all-gathers, "cc" = ncfw collective_compute (single AllReduce + AllGather).
# Measured on HW (interleaved A/B, 33-rep NEFFs): cc 157.5us/rep,
# baseline 183.7us/rep, rdma 232.6us/rep (Q7 SWDGE desc-gen per
# broadcast dominates) -> cc.
COMM = "cc"

# split the big fact loads across two HWDGE queues (SP + Pool):
# measured -13us/rep vs single-queue in an interleaved A/B (per-queue
# descriptor throughput, not HBM, was the limiter), identical numerics
DQ = True

# per-broadcast semaphore increments: local_sem +16 per broadcast,
# remote_sem +2 (16 lanes / 8 slots) per delivery; 7 peers per gather
LSEM_PER_REP = 16 * (NCORES - 1)
RSEM_PER_REP = 2 * (NCORES - 1)

_CACHE = {}


def _dummy_out(nc, rows, out_ep, out_st):
    d1 = rows.tile([1, H], F32, tag="rows")
    d2 = rows.tile([1, H + 2], F32, tag="row770")
    nc.vector.memset(d1, 0.0)
    nc.vector.memset(d2, 0.0)
    nc.gpsimd.dma_start(out=out_ep, in_=d1)
    nc.gpsimd.dma_start(out=out_st, in_=d2)


def _allgather(nc, pools, src, dst_par, rsem, lsem, rep, tag):
    """All-gather src [128, W] into dst_par [128, 8, W] (slot 0 = self,
    slot k = phys-peer XOR k). Returns the trigger instruction. Caller
    attaches the arrival wait (rsem >= RSEM_PER_REP*(rep+1)) to the
    first consumer of dst_par."""
    c0 = nc.vector.tensor_copy(dst_par[:, 0, :], src)
    first = None
    for k in range(1, NCORES):
        rdests = [None] * NCORES
        rdests[k] = (0, k)
        p = nc.gpsimd.remote_dma_broadcast(
            out_ap=dst_par[:, k, :], in_ap=src,
            remote_sem=rsem, local_sem=lsem, rdests=rdests,
        )
        if first is None:
            first = p
    if rep == 0:
        # all-cores-entered barrier before the first remote send of the
        # kernel (prelude AllGather inc; overlaps the input load)
        nc._bir_kernel_barrier_sem_replica_groups.extend(set(g) for g in RG)
        pools["_waits"].append(
            (first, nc._bir_kernel_barrier_sem, nc.bir_kernel_barrier_sem_inc))
    nc.gpsimd.trigger_dma(count=None)
    return c0


def _body(nc, tc, ctx, factT, factn, epT, epn, w, wt, out_ep, out_st, stage=999, pools=None, sim1=False, comm=None, dq=None):
    if pools is None:
        pools = {}
    if comm is None:
        comm = COMM
    if dq is None:
        dq = DQ
    rep = pools.setdefault("_rep", 0)
    pools["_rep"] = rep + 1
    pools.setdefault("_waits", [])
    def pool(name, **kw):
        if name not in pools:
            pools[name] = ctx.enter_context(tc.tile_pool(name=name, **kw))
        return pools[name]
    big = pool("big", bufs=1)
    rows = pool("rows", bufs=2)
    tiny = pool("tiny", bufs=1)
    psA = pool("psA", bufs=1, space="PSUM")
    psE = pool("psE", bufs=2, space="PSUM")
    psT = pool("psT", bufs=1, space="PSUM")
    if "consts" not in pools:
        ones11_t = tiny.tile([1, 1], F32, tag="ones11")
        nc.vector.memset(ones11_t, 1.0)
        ones_col_t = tiny.tile([128, 1], F32, tag="ones_col")
        nc.vector.memset(ones_col_t, 1.0)
        ident_d = nc.inline_tensor(np.eye(128, dtype=np.float32), name="ident128")
        ident_t = tiny.tile([128, 128], F32, tag="ident128")
        nc.sync.dma_start(out=ident_t, in_=ident_d.ap())
        onesr_t = tiny.tile([128, 1], F32R, tag="ones_colr")
        nc.vector.tensor_copy(onesr_t, ones_col_t)
        sems = (nc.alloc_semaphore("ag1_r"), nc.alloc_semaphore("ag1_l"),
                nc.alloc_semaphore("ag2_r"), nc.alloc_semaphore("ag2_l"))
        pools["consts"] = (ones11_t, ones_col_t, ident_t, onesr_t)
        pools["sems"] = sems
    ones11, ones_col, ident, ones_colr = pools["consts"]
    rsem1, lsem1, rsem2, lsem2 = pools["sems"]

    # ---- big input loads. factT streams in as fp32-bit-exact float32r
    # quarter-chunks on the SP HWDGE ring straight into the resident PE
    # matvec tile (f32r storage is plain fp32 bytes; the rounding is a PE
    # matmul mode), and each quarter is reduced in place for the exact
    # column max. fnorm follows on the same ring.
    QRT = NS // 4
    fst = pool("fst", bufs=6)  # holds the 6 resident wt chunks
    factT_sb = big.tile([128, HC, NS], F32R, tag="factT")
    factT_d = factT.rearrange("(c p) (u n) -> p c u n", p=128, u=4)
    fQh = tiny.tile([128, HC, 4], F32, tag="fQh")
    _fact_dmas = []
    for c in range(HC):
        for u in range(4):
            sl = factT_sb[:, c, u * QRT : (u + 1) * QRT]
            eng = nc.gpsimd if (dq and (c * 4 + u) % 2) else nc.sync
            _fact_dmas.append(eng.dma_start(out=sl, in_=factT_d[:, c, u, :]))
            nc.vector.reduce_max(fQh[:, c, u : u + 1], sl, axis=mybir.AxisListType.X)

    # ---- small input loads (ACT HWDGE ring, parallel with the SP ring)
    smw = pool("smw", bufs=2)
    smept = pool("smept", bufs=2)
    smep = pool("smep", bufs=2)
    w_d = w.rearrange("(c p) j -> p c j", p=128)
    wt_d = wt.rearrange("(c p) j -> p c j", p=128)
    epT_d = epT.rearrange("(c p) m -> p c m", p=128)
    epn_d = epn.rearrange("(j p) h -> p j h", p=128)
    w_ch, wt_ch, epT_ch, ep_ch = [], [], [], []
    _gate = _fact_dmas[15].ins
    _prev_small = None
    def _small_dma(dst, src):
        nonlocal _prev_small
        d = nc.scalar.dma_start(out=dst, in_=src)
        if _prev_small is None:
            tile.add_dep_helper(d.ins, _gate, sync=True, reason="factT first")
        else:
            tile.add_dep_helper(d.ins, _prev_small, sync=False, reason="ACT ring order")
        _prev_small = d.ins
        return d
    for c in range(HC):
        t = smw.tile([128, H], F32R, tag="w")
        _small_dma(t, w_d[:, c, :])
        w_ch.append(t)
    for c in range(HC):
        t = smept.tile([128, MS], F32R, tag="epT")
        _small_dma(t, epT_d[:, c, :])
        epT_ch.append(t)
    for jj in range(MQ):
        t = smep.tile([128, H], F32R, tag="ep")
        _small_dma(t, epn_d[:, jj, :])
        ep_ch.append(t)
    for c in range(HC):
        t = fst.tile([128, H], F32R, tag="fst")
        _small_dma(t, wt_d[:, c, :])
        wt_ch.append(t)
    # fnorm on the SP ring right after factT (needed only by the final
    # weighted sum; the SBUF-to-SBUF gathers don't contend with it)
    fnorm_sb = big.tile([128, NT, H], BF16, tag="fnorm")
    _fn_d = factn.rearrange("(j p) h -> p j h", p=128)
    nc.sync.dma_start(out=fnorm_sb[:, 0 : NT // 2, :], in_=_fn_d[:, 0 : NT // 2, :])
    (nc.gpsimd if dq else nc.sync).dma_start(
        out=fnorm_sb[:, NT // 2 : NT, :], in_=_fn_d[:, NT // 2 : NT, :])

    if stage < 1:
        _dummy_out(nc, rows, out_ep, out_st)
        return
    def _warm(n, c=0, off=0):
        for _ in range(n):
            ps_w = psE.tile([1, 512], F32, tag="psE")
            nc.tensor.matmul(ps_w, ones_colr, factT_sb[:, c, off : off + 512],
                             start=True, stop=True)
    for k in range(12):
        _warm(W1, c=k // 2, off=(k % 2) * 2048)
    # ---- pass 1 finish: combine quarter-chunk maxes into fQl [128, HC]
    fQl = tiny.tile([128, HC], F32, tag="fQl")
    r1 = nc.vector.reduce_max(fQl, fQh, axis=mybir.AxisListType.X)
    if rep > 0 and comm != "cc" and not sim1:
        pools["_waits"].append((r1, lsem1, LSEM_PER_REP * rep))
    _warm(W2, c=HC - 1, off=2048)

    if stage < 11:
        _dummy_out(nc, rows, out_ep, out_st)
        return
    # ---- AG1: all-gather the local column maxes, then reduce (max is
    # permutation-invariant across slots)
    par = rep % 2
    fQg = tiny.tile([128, HC], F32R, tag="fQg")
    if comm == "cc" and not sim1:
        dram = pool("dram", bufs=1, space="DRAM")
        cc1_in = dram.tile([H], F32, tag="cc1i")
        cc1_out = dram.tile([H], F32, tag="cc1o")
        nc.gpsimd.dma_start(out=cc1_in[:].rearrange("(c p) -> p c", p=128), in_=fQl)
        nc.gpsimd.collective_compute(
            "AllReduce", mybir.AluOpType.max, replica_groups=RG,
            ins=[cc1_in.opt()], outs=[cc1_out.opt()],
        )
        nc.gpsimd.dma_start(out=fQg, in_=cc1_out[:].rearrange("(c p) -> p c", p=128))
    else:
        Y1 = tiny.tile([128, 2, NCORES, HC], F32, tag="Y1")
        if sim1:
            for k in range(NCORES):
                nc.vector.tensor_copy(Y1[:, par, k, :], fQl)
        else:
            _allgather(nc, pools, fQl, Y1[:, par], rsem1, lsem1, rep, "ag1")
        m4 = tiny.tile([128, 4, HC], F32, tag="m4")
        m2 = tiny.tile([128, 2, HC], F32, tag="m2")
        x1 = nc.vector.tensor_tensor(out=m4, in0=Y1[:, par, 0:4, :], in1=Y1[:, par, 4:8, :],
                                     op=mybir.AluOpType.max)
        if not sim1:
            pools["_waits"].append((x1, rsem1, RSEM_PER_REP * (rep + 1)))
        nc.vector.tensor_tensor(out=m2, in0=m4[:, 0:2, :], in1=m4[:, 2:4, :],
                                op=mybir.AluOpType.max)
        nc.vector.tensor_tensor(out=fQg, in0=m2[:, 0, :], in1=m2[:, 1, :],
                                op=mybir.AluOpType.max)

    if stage < 12:
        _dummy_out(nc, rows, out_ep, out_st)
        return
    regions768 = [(0, 512), (512, 256)]

    # ---- qw = fact_Q @ W  (contraction over h; lhsT = fQg chunks)
    ps_qw = psA.tile([1, H], F32, tag="psA")
    for c in range(HC):
        for n0, nn in regions768:
            nc.tensor.matmul(
                ps_qw[:, n0 : n0 + nn], fQg[:, c : c + 1], w_ch[c][:, n0 : n0 + nn],
                start=(c == 0), stop=(c == HC - 1),
            )
    qw_row = rows.tile([1, H], F32, tag="rows")
    nc.scalar.activation(out=qw_row, in_=ps_qw, func=mybir.ActivationFunctionType.Copy)

    # transpose qw row -> [128, HC] via K=1 matmuls against ones
    qwT = tiny.tile([128, HC], F32R, tag="qwT")
    ps_t = psT.tile([128, HC], F32, tag="psT")
    for c in range(HC):
        nc.tensor.matmul(ps_t[:, c : c + 1], qw_row[:, c * 128 : (c + 1) * 128], ones11,
                         start=True, stop=True)
    nc.vector.tensor_copy(qwT, ps_t)

    if stage < 13:
        _dummy_out(nc, rows, out_ep, out_st)
        return
    # ---- f2e local logits [1, MS]
    ps_f2e = psA.tile([1, MS], F32, tag="psA")
    for c in range(HC):
        nc.tensor.matmul(
            ps_f2e, qwT[:, c : c + 1], epT_ch[c],
            start=(c == 0), stop=(c == HC - 1),
        )
    # local softmax pieces
    cmax_l = tiny.tile([1, 1], F32, tag="cmax_l")
    neg_cmax = tiny.tile([1, 1], F32, tag="neg_cmax")
    nc.vector.reduce_max(cmax_l, ps_f2e, axis=mybir.AxisListType.X)
    nc.vector.tensor_reduce(neg_cmax, ps_f2e, axis=mybir.AxisListType.X,
                            op=mybir.AluOpType.max, negate=True)
    p_loc = rows.tile([1, MS], F32, tag="rows")
    s_loc = tiny.tile([1, 1], F32, tag="s_loc")
    nc.scalar.activation(
        out=p_loc, in_=ps_f2e, func=mybir.ActivationFunctionType.Exp,
        bias=neg_cmax, scale=1.0, accum_out=s_loc,
    )

    # transpose p_loc [1,512] -> pT [128, MQ] (m = jj*128 + p)
    pT = tiny.tile([128, MQ], F32R, tag="pT")
    ps_t2 = psT.tile([128, MQ], F32, tag="psT")
    for jj in range(MQ):
        nc.tensor.matmul(ps_t2[:, jj : jj + 1], p_loc[:, jj * 128 : (jj + 1) * 128], ones11,
                         start=True, stop=True)
    nc.vector.tensor_copy(pT, ps_t2)

    if stage < 14:
        _dummy_out(nc, rows, out_ep, out_st)
        return
    # ---- accep_local = p_loc @ ep_local  [1, H]
    ps_accep = psA.tile([1, H], F32, tag="psA")
    for jj in range(MQ):
        for n0, nn in regions768:
            nc.tensor.matmul(
                ps_accep[:, n0 : n0 + nn], pT[:, jj : jj + 1], ep_ch[jj][:, n0 : n0 + nn],
                start=(jj == 0), stop=(jj == MQ - 1),
            )

    if stage < 2:
        r = rows.tile([1, H], F32, tag="rows")
        nc.vector.tensor_copy(r, ps_accep)
        nc.gpsimd.dma_start(out=out_ep, in_=r)
        d2 = rows.tile([1, H + 2], F32, tag="row770")
        nc.vector.memset(d2, 0.0)
        nc.gpsimd.dma_start(out=out_st, in_=d2)
        return
    # ---- AG2: pack [accep cols | cmax | s] as a [128, 8] column payload
    # (accep transposed to h-column form, stats on partition 0), gather,
    # and combine locally — identical result on every core.
    accep_row = rows.tile([1, H], F32, tag="rows")
    nc.scalar.activation(out=accep_row, in_=ps_accep,
                         func=mybir.ActivationFunctionType.Copy)
    ps_t5 = psT.tile([128, HC], F32, tag="psT")
    for c in range(HC):
        nc.tensor.matmul(ps_t5[:, c : c + 1], accep_row[:, c * 128 : (c + 1) * 128],
                         ones11, start=True, stop=True)
    pay = tiny.tile([128, NCORES], F32, tag="pay")
    pw = nc.vector.tensor_copy(pay[:, 0:HC], ps_t5)
    if rep > 0 and comm != "cc" and not sim1:
        pools["_waits"].append((pw, lsem2, LSEM_PER_REP * rep))
    # chain the other pay writers behind pw so the lsem2 drain wait
    # covers every overwrite of the in-flight send buffer
    ms = nc.vector.memset(pay[:, HC : HC + 2], 0.0)
    tile.add_dep_helper(ms.ins, pw.ins, sync=False, reason="pay WAR order")
    for wi in (
        nc.vector.tensor_copy(pay[0:1, HC : HC + 1], cmax_l),
        nc.vector.tensor_copy(pay[0:1, HC + 1 : HC + 2], s_loc),
    ):
        tile.add_dep_helper(wi.ins, ms.ins, sync=False, reason="stats after memset")
    Y2 = tiny.tile([128, 2, NCORES, NCORES], F32, tag="Y2")
    if comm == "cc" and not sim1:
        dram = pool("dram", bufs=1, space="DRAM")
        cc2_in = dram.tile([128 * NCORES], F32, tag="cc2i")
        cc2_out = dram.tile([NCORES * 128 * NCORES], F32, tag="cc2o")
        nc.gpsimd.dma_start(out=cc2_in[:].rearrange("(p c) -> p c", p=128), in_=pay)
        nc.gpsimd.collective_compute(
            "AllGather", mybir.AluOpType.bypass, replica_groups=RG,
            ins=[cc2_in.opt()], outs=[cc2_out.opt()],
        )
        nc.gpsimd.dma_start(
            out=Y2[:, par],
            in_=cc2_out[:].rearrange("(r p c) -> p r c", p=128, c=NCORES))
    elif sim1:
        for k in range(NCORES):
            nc.vector.tensor_copy(Y2[:, par, k, :], pay)
    else:
        _allgather(nc, pools, pay, Y2[:, par], rsem2, lsem2, rep, "ag2")
    _warm(W3, c=0, off=0)

    # combine: coef_r = exp(cmax_r - max_r cmax_r); D = sum coef_r * s_r;
    # ep_col = sum (coef_r / D) * accep_col_r   (slot-order invariant)
    cmrow = tiny.tile([1, NCORES], F32, tag="cmrow")
    x2w = nc.vector.tensor_copy(
        cmrow, Y2[0:1, par, :, HC : HC + 1].rearrange("p r one -> p (r one)"))
    if not sim1 and comm != "cc":
        pools["_waits"].append((x2w, rsem2, RSEM_PER_REP * (rep + 1)))
    srow = tiny.tile([1, NCORES], F32, tag="srow")
    sr = nc.vector.tensor_copy(
        srow, Y2[0:1, par, :, HC + 1 : HC + 2].rearrange("p r one -> p (r one)"))
    tile.add_dep_helper(sr.ins, x2w.ins, sync=False, reason="after AG2 arrival wait")
    negc2 = tiny.tile([1, 1], F32, tag="negc2")
    nc.vector.tensor_reduce(negc2, cmrow, axis=mybir.AxisListType.X,
                            op=mybir.AluOpType.max, negate=True)
    coef = tiny.tile([1, NCORES], F32, tag="coef")
    nc.scalar.activation(out=coef, in_=cmrow, func=mybir.ActivationFunctionType.Exp,
                         bias=negc2, scale=1.0)
    prod = tiny.tile([1, NCORES], F32, tag="prod")
    nc.vector.tensor_tensor(out=prod, in0=coef, in1=srow,
                            op=mybir.AluOpType.mult)
    D = tiny.tile([1, 1], F32, tag="D")
    nc.vector.reduce_sum(D, prod, axis=mybir.AxisListType.X)
    Sinv = tiny.tile([1, 1], F32, tag="Sinv")
    nc.vector.reciprocal(Sinv, D)
    csc = tiny.tile([1, NCORES], F32, tag="csc")
    nc.vector.tensor_scalar_mul(csc, coef, Sinv)
    # broadcast csc across partitions via ones-matmul
    ps_cb = psT.tile([128, NCORES], F32, tag="psT2")
    nc.tensor.matmul(ps_cb, ones_col[0:1, :].broadcast_to([1, 128]), csc,
                     start=True, stop=True)
    cscB = tiny.tile([128, NCORES], F32, tag="cscB")
    nc.vector.tensor_copy(cscB, ps_cb)
    # ep_col = sum_r cscB[:, r] * accep_col_r
    sc = tiny.tile([128, NCORES, HC], F32, tag="sc")
    for r in range(NCORES):
        nc.vector.tensor_scalar_mul(sc[:, r, :], Y2[:, par, r, 0:HC], cscB[:, r : r + 1])
    sa = tiny.tile([128, 4, HC], F32, tag="sa")
    sb = tiny.tile([128, 2, HC], F32, tag="sb")
    ep_col = tiny.tile([128, HC], F32, tag="ep_col")
    nc.vector.tensor_tensor(out=sa, in0=sc[:, 0:4, :], in1=sc[:, 4:8, :],
                            op=mybir.AluOpType.add)
    nc.vector.tensor_tensor(out=sb, in0=sa[:, 0:2, :], in1=sa[:, 2:4, :],
                            op=mybir.AluOpType.add)
    nc.vector.tensor_tensor(out=ep_col, in0=sb[:, 0, :], in1=sb[:, 1, :],
                            op=mybir.AluOpType.add)
    nc.gpsimd.dma_start(out=out_ep.rearrange("a (c p) -> p (a c)", p=128), in_=ep_col)
    ep_colr = tiny.tile([128, HC], F32R, tag="ep_colr")
    nc.vector.tensor_copy(ep_colr, ep_col)

    # ---- v = W @ ep_.T  as a row: v_row = ep_ @ W.T  [1, H]
    ps_v = psA.tile([1, H], F32, tag="psA")
    for c in range(HC):
        for n0, nn in regions768:
            nc.tensor.matmul(
                ps_v[:, n0 : n0 + nn], ep_colr[:, c : c + 1], wt_ch[c][:, n0 : n0 + nn],
                start=(c == 0), stop=(c == HC - 1),
            )
    v_row = rows.tile([1, H], F32, tag="rows")
    nc.scalar.activation(out=v_row, in_=ps_v, func=mybir.ActivationFunctionType.Copy)
    vT = tiny.tile([128, HC], F32R, tag="vT")
    ps_t4 = psT.tile([128, HC], F32, tag="psT")
    for c in range(HC):
        nc.tensor.matmul(ps_t4[:, c : c + 1], v_row[:, c * 128 : (c + 1) * 128], ones11,
                         start=True, stop=True)
    nc.vector.tensor_copy(vT, ps_t4)

    if stage < 3:
        d2 = rows.tile([1, H + 2], F32, tag="row770")
        nc.vector.memset(d2, 0.0)
        nc.gpsimd.dma_start(out=out_st, in_=d2)
        return
    # ---- e2f local logits [1, NS] in float32r, transposed to [128, NT]
    # via K=1 matmuls (n = j*128 + p)
    x2 = psT.tile([128, NT], F32, tag="psX")
    _pend = None
    for b in range(NB):
        ps_e2f = psE.tile([1, 512], F32, tag="psE")
        for c in range(HC):
            nc.tensor.matmul(
                ps_e2f,
                vT[:, c : c + 1],
                factT_sb[:, c, b * 512 : (b + 1) * 512],
                start=(c == 0), stop=(c == HC - 1),
            )
        if _pend is not None:
            pb, prow = _pend
            for jj in range(4):
                j = pb * 4 + jj
                nc.tensor.matmul(x2[:, j : j + 1], prow[:, jj * 128 : (jj + 1) * 128],
                                 ones11, start=True, stop=True)
        e2f_row = rows.tile([1, 512], F32, tag="rows")
        nc.scalar.activation(out=e2f_row, in_=ps_e2f,
                             func=mybir.ActivationFunctionType.Copy)
        _pend = (b, e2f_row)
    pb, prow = _pend
    for jj in range(4):
        j = pb * 4 + jj
        nc.tensor.matmul(x2[:, j : j + 1], prow[:, jj * 128 : (jj + 1) * 128],
                         ones11, start=True, stop=True)

    # ---- e2f softmax partials (n = p*NT + j)
    m_p = tiny.tile([128, 1], F32, tag="m_p")
    nc.vector.reduce_max(m_p, x2, axis=mybir.AxisListType.X)
    ps_mr = psT.tile([1, 128], F32, tag="psT")
    nc.tensor.matmul(ps_mr, m_p, ident, start=True, stop=True)
    Cg = tiny.tile([1, 1], F32, tag="Cg")
    negCg = tiny.tile([1, 1], F32, tag="negCg")
    nc.vector.reduce_max(Cg, ps_mr, axis=mybir.AxisListType.X)
    nc.vector.tensor_reduce(negCg, ps_mr, axis=mybir.AxisListType.X,
                            op=mybir.AluOpType.max, negate=True)
    ps_cb2 = psT.tile([128, 1], F32, tag="psT2")
    nc.tensor.matmul(ps_cb2, ones_col[0:1, :].broadcast_to([1, 128]), negCg,
                     start=True, stop=True)
    negC = tiny.tile([128, 1], F32, tag="negC")
    nc.vector.tensor_copy(negC, ps_cb2)
    s_ps = tiny.tile([128, 1], F32, tag="s_ps")
    p2bf = tiny.tile([128, NT], BF16, tag="p2bf")
    nc.scalar.activation(
        out=p2bf, in_=x2, func=mybir.ActivationFunctionType.Exp,
        bias=negC, scale=1.0, accum_out=s_ps,
    )
    ps_S = psT.tile([1, 1], F32, tag="psT3")
    nc.tensor.matmul(ps_S, s_ps, ones_col, start=True, stop=True)

    if stage < 4:
        d2 = rows.tile([1, H + 2], F32, tag="row770")
        nc.vector.memset(d2, 0.0)
        nc.gpsimd.dma_start(out=out_st, in_=d2)
        return
    # ---- acc = sum_n p_n * fact[n, :]  (bf16 matmuls, fp32 accumulate)
    ps_acc = psA.tile([1, H], F32, tag="psA")
    for n0, nn in regions768:
        for j in range(NT):
            nc.tensor.matmul(
                ps_acc[:, n0 : n0 + nn], p2bf[:, j : j + 1], fnorm_sb[:, j, n0 : n0 + nn],
                start=(j == 0), stop=(j == NT - 1),
            )

    # ---- stats out: [C, S, acc]
    st_row = rows.tile([1, H + 2], F32, tag="row770")
    nc.vector.tensor_copy(st_row[:, 0:1], Cg)
    nc.vector.tensor_copy(st_row[:, 1:2], ps_S)
    nc.scalar.activation(out=st_row[:, 2 : H + 2], in_=ps_acc,
                         func=mybir.ActivationFunctionType.Copy)
    nc.gpsimd.dma_start(out=out_st, in_=st_row)


def build(stage=999, reps=1, sim1=False, comm=None, dq=None):
    nc = bacc.Bacc("TRN2", target_bir_lowering=False, debug=False,
                   num_devices=1 if sim1 else NCORES)
    factT = nc.dram_tensor("factT", [H, NS], F32R, kind="ExternalInput").ap()
    factn = nc.dram_tensor("factn", [NS, H], BF16, kind="ExternalInput").ap()
    epT = nc.dram_tensor("epT", [H, MS], F32R, kind="ExternalInput").ap()
    epn = nc.dram_tensor("epn", [MS, H], F32R, kind="ExternalInput").ap()
    w = nc.dram_tensor("w", [H, H], F32R, kind="ExternalInput").ap()
    wt = nc.dram_tensor("wt", [H, H], F32R, kind="ExternalInput").ap()
    out_ep = nc.dram_tensor("out_ep", [1, H], F32, kind="ExternalOutput").ap()
    out_st = nc.dram_tensor("out_st", [1, H + 2], F32, kind="ExternalOutput").ap()
    from contextlib import ExitStack

    pools = {}
    with tile.TileContext(nc) as tc:
        with ExitStack() as ctx:
            for _ in range(reps):
                _body(nc, tc, ctx, factT, factn, epT, epn, w, wt, out_ep, out_st,
                      stage=stage, pools=pools, sim1=sim1, comm=comm, dq=dq)
    # attach remote-arrival / drain waits after tile scheduling (the
    # scheduler's single-core sim cannot satisfy remote increments);
    # compile()'s generate_event_semaphores legalizes multi-wait insts
    for inst, sem, thr in pools.get("_waits", []):
        inst.wait_op(sem, thr, "sem-ge", check=False)
    nc.compile()
    return nc


def make_in_maps(fact, elements_p, weights):
    fact = np.asarray(fact, np.float32)
    elements_p = np.asarray(elements_p, np.float32)
    weights = np.asarray(weights, np.float32)
    wt = np.ascontiguousarray(weights.T)
    in_maps = []
    for i in range(NCORES):
        fs = fact[i * NS : (i + 1) * NS]
        es = elements_p[i * MS : (i + 1) * MS]
        in_maps.append({
            "factT": np.ascontiguousarray(fs.T),
            "factn": fs.astype(ml_dtypes.bfloat16),
            "epT": np.ascontiguousarray(es.T),
            "epn": np.ascontiguousarray(es),
            "w": weights,
            "wt": wt,
        })
    return in_maps


def combine(results):
    st = np.stack([np.asarray(results[i]["out_st"][0], np.float64) for i in range(NCORES)])
    C = st[:, 0]
    S = st[:, 1]
    acc = st[:, 2:]
    Cg = C.max()
    coef = np.exp(C - Cg)
    fact_ = (coef[:, None] * acc).sum(0, keepdims=True) / (coef * S).sum()
    ep_ = np.asarray(results[0]["out_ep"], np.float32)
    return fact_.astype(np.float32), ep_


def kernel(fact, elements_p, weights, **run_kwargs):
    if "nc" not in _CACHE:
        _CACHE["nc"] = build()
    nc = _CACHE["nc"]
    in_maps = make_in_maps(fact, elements_p, weights)
    res = bass_utils.run_bass_kernel_spmd(
        nc, in_maps, core_ids=list(range(NCORES)), **run_kwargs
    )
    _CACHE["last_result"] = res
    return combine(res.results)


def build_sim1():
    return build(sim1=True)



# revision 28
# speedup vs baseline: 1.2754x; 1.2754x over previous
"""Trainium2 Bass kernel for nn_AggregationLayer (pooling).

Computation (reference):
    fact: [N,H], elements_p: [M,H], weights: [H,H]   (N=32768, M=4096, H=768)
    fact_Q = max(fact, axis=0)                     # [1,H]
    f2e = softmax(fact_Q @ W @ ep.T)               # [1,M]
    ep_ = f2e @ ep                                 # [1,H]
    e2f = softmax_n(fact @ W @ ep_.T)              # [N,1]
    fact_ = e2f.T @ fact                           # [1,H]
    returns (fact_, ep_)

Distribution over 8 NeuronCores:
    fact sharded along N (4096 rows per core, sent in 2 layouts:
    float32r transposed [H, N/8] DMA'd straight into the resident PE
    matvec tile — f32r storage is plain fp32 bytes, giving full-rate
    PE matmuls while the col-max reduce stays fp32-exact — and bf16
    row-major for the final weighted row-sum), elements_p sharded
    along M (512 rows per core, both layouts f32r), weights replicated
    f32r (W and W.T). Cross-core: one AllReduce(max) for fact_Q and
    one AllGather of a [128, 8] column-form payload (accep transposed
    to h-columns + the two softmax stats), combined on every core with
    slot-order-invariant math. A remote_dma_broadcast SBUF-to-SBUF
    all-gather path exists behind COMM="rdma" (correct on HW, but the
    per-broadcast Q7 SWDGE descriptor generation costs more per rep
    than ncfw collectives). The final e2f softmax partials (one scalar
    max, one scalar sum, one [768] weighted sum per core) are combined
    on host.

Precision envelope (measured end-to-end on the seed data, tol 2e-2):
    The f2e softmax logits are ~234 in magnitude with a top-2 gap of
    1.78, and p1 = [0.856, 0.144, ~0] — NOT one-hot.  Any bf16
    rounding upstream of those logits (fact col-max, W, or ep) shifts
    the logits by O(0.3-0.9) and blows fact_ to 4e-2..1.4e-1.  So
    fact/W/ep must stay fp32-stored (f32r matmul mode is fine); only
    the final weighted row-sum source (fnorm) tolerates bf16
    (1.8e-3).  fdt="bf16" exists but fails tolerance — do not ship.

Timing methodology (axon): each RPC dispatch costs ~90 ms with ~2 ms
    jitter while 32 extra in-NEFF reps add only ~1.5-4 ms, so per-rep
    time needs hundreds of alternated samples and trimmed means (see
    test.py).  Absolute per-rep throughput drifts 2-3x over tens of
    minutes on this shared device; only interleaved A/B comparisons
    (ab.py) are decision-grade.  warm=(3,10,8) vs (3,25,18) measured
    as a wash (-5+/-7 us).
"""
import sys

sys.path.insert(0, "/opt/trn_rl_repo")

import numpy as np
import ml_dtypes

import concourse.bass as bass
import concourse.tile as tile
import concourse.mybir as mybir
from concourse import bacc, bass_utils

N, M, H = 32768, 4096, 768
NCORES = 8
NS = N // NCORES          # 4096 fact rows per core
MS = M // NCORES          # 512 ep rows per core
HC = H // 128             # 6 h-chunks of 128
NB = NS // 512            # 8 psum banks for e2f
NT = NS // 128            # 32 acc tiles
MQ = MS // 128            # 4 accep tiles

F32 = mybir.dt.float32
F32R = mybir.dt.float32r
BF16 = mybir.dt.bfloat16
RG = [list(range(NCORES))]

# PE clock-keeper matmul counts: W1 groups during the factT load,
# W2 during the AG1 (col-max exchange) flight, W3 during AG2.
W1, W2, W3 = 3, 25, 18

# cross-core exchange implementation: "rdma" = remote_dma_broadcast
# all-gathers, "cc" = ncfw collective_compute (single AllReduce + AllGather).
# Measured on HW (interleaved A/B, 33-rep NEFFs): cc 157.5us/rep,
# baseline 183.7us/rep, rdma 232.6us/rep (Q7 SWDGE desc-gen per
# broadcast dominates) -> cc.
COMM = "cc"

# factT storage/matvec dtype: "f32r" (exact col-max, f32r matvec) or
# "bf16" (half the dominant DMA; col-max+matvec in bf16).
import os
FDT = os.environ.get("KFDT", "f32r")

# split the big fact loads across two HWDGE queues (SP + Pool):
# measured -13us/rep vs single-queue in an interleaved A/B (per-queue
# descriptor throughput, not HBM, was the limiter), identical numerics
DQ = True

# per-broadcast semaphore increments: local_sem +16 per broadcast,
# remote_sem +2 (16 lanes / 8 slots) per delivery; 7 peers per gather
LSEM_PER_REP = 16 * (NCORES - 1)
RSEM_PER_REP = 2 * (NCORES - 1)

_CACHE = {}


def _dummy_out(nc, rows, out_ep, out_st):
    d1 = rows.tile([1, H], F32, tag="rows")
    d2 = rows.tile([1, H + 2], F32, tag="row770")
    nc.vector.memset(d1, 0.0)
    nc.vector.memset(d2, 0.0)
    nc.gpsimd.dma_start(out=out_ep, in_=d1)
    nc.gpsimd.dma_start(out=out_st, in_=d2)


def _allgather(nc, pools, src, dst_par, rsem, lsem, rep, tag):
    """All-gather src [128, W] into dst_par [128, 8, W] (slot 0 = self,
    slot k = phys-peer XOR k). Returns the trigger instruction. Caller
    attaches the arrival wait (rsem >= RSEM_PER_REP*(rep+1)) to the
    first consumer of dst_par."""
    c0 = nc.vector.tensor_copy(dst_par[:, 0, :], src)
    first = None
    for k in range(1, NCORES):
        rdests = [None] * NCORES
        rdests[k] = (0, k)
        p = nc.gpsimd.remote_dma_broadcast(
            out_ap=dst_par[:, k, :], in_ap=src,
            remote_sem=rsem, local_sem=lsem, rdests=rdests,
        )
        if first is None:
            first = p
    if rep == 0:
        # all-cores-entered barrier before the first remote send of the
        # kernel (prelude AllGather inc; overlaps the input load)
        nc._bir_kernel_barrier_sem_replica_groups.extend(set(g) for g in RG)
        pools["_waits"].append(
            (first, nc._bir_kernel_barrier_sem, nc.bir_kernel_barrier_sem_inc))
    nc.gpsimd.trigger_dma(count=None)
    return c0


def _body(nc, tc, ctx, factT, factn, epT, epn, w, wt, out_ep, out_st, stage=999, pools=None, sim1=False, comm=None, dq=None, fdt=F32R, warm=None, ccq=None):
    w1, w2, w3 = warm if warm is not None else (W1, W2, W3)
    # ccq='act': stage collective payloads and kernel outputs through the
    # ACT HWDGE ring instead of gpsimd SWDGE (skips per-DMA descriptor
    # generation on the Q7 ucode path).
    ceng = nc.scalar if ccq == "act" else nc.gpsimd
    if pools is None:
        pools = {}
    if comm is None:
        comm = COMM
    if dq is None:
        dq = DQ
    rep = pools.setdefault("_rep", 0)
    pools["_rep"] = rep + 1
    pools.setdefault("_waits", [])
    def pool(name, **kw):
        if name not in pools:
            pools[name] = ctx.enter_context(tc.tile_pool(name=name, **kw))
        return pools[name]
    big = pool("big", bufs=1)
    rows = pool("rows", bufs=2)
    tiny = pool("tiny", bufs=1)
    psA = pool("psA", bufs=1, space="PSUM")
    psE = pool("psE", bufs=2, space="PSUM")
    psT = pool("psT", bufs=1, space="PSUM")
    if "consts" not in pools:
        ones11_t = tiny.tile([1, 1], F32, tag="ones11")
        nc.vector.memset(ones11_t, 1.0)
        ones_col_t = tiny.tile([128, 1], F32, tag="ones_col")
        nc.vector.memset(ones_col_t, 1.0)
        ident_d = nc.inline_tensor(np.eye(128, dtype=np.float32), name="ident128")
        ident_t = tiny.tile([128, 128], F32, tag="ident128")
        nc.sync.dma_start(out=ident_t, in_=ident_d.ap())
        onesr_t = tiny.tile([128, 1], fdt, tag="ones_colr")
        nc.vector.tensor_copy(onesr_t, ones_col_t)
        sems = (nc.alloc_semaphore("ag1_r"), nc.alloc_semaphore("ag1_l"),
                nc.alloc_semaphore("ag2_r"), nc.alloc_semaphore("ag2_l"))
        pools["consts"] = (ones11_t, ones_col_t, ident_t, onesr_t)
        pools["sems"] = sems
    ones11, ones_col, ident, ones_colr = pools["consts"]
    rsem1, lsem1, rsem2, lsem2 = pools["sems"]

    # ---- big input loads. factT streams in as fp32-bit-exact float32r
    # quarter-chunks on the SP HWDGE ring straight into the resident PE
    # matvec tile (f32r storage is plain fp32 bytes; the rounding is a PE
    # matmul mode), and each quarter is reduced in place for the exact
    # column max. fnorm follows on the same ring.
    QRT = NS // 4
    fst = pool("fst", bufs=6)  # holds the 6 resident wt chunks
    factT_sb = big.tile([128, HC, NS], fdt, tag="factT")
    factT_d = factT.rearrange("(c p) (u n) -> p c u n", p=128, u=4)
    fQh = tiny.tile([128, HC, 4], F32, tag="fQh")
    _fact_dmas = []
    for c in range(HC):
        for u in range(4):
            sl = factT_sb[:, c, u * QRT : (u + 1) * QRT]
            eng = nc.gpsimd if (dq and (c * 4 + u) % 2) else nc.sync
            _fact_dmas.append(eng.dma_start(out=sl, in_=factT_d[:, c, u, :]))
            nc.vector.reduce_max(fQh[:, c, u : u + 1], sl, axis=mybir.AxisListType.X)

    # ---- small input loads (ACT HWDGE ring, parallel with the SP ring)
    smw = pool("smw", bufs=2)
    smept = pool("smept", bufs=2)
    smep = pool("smep", bufs=2)
    w_d = w.rearrange("(c p) j -> p c j", p=128)
    wt_d = wt.rearrange("(c p) j -> p c j", p=128)
    epT_d = epT.rearrange("(c p) m -> p c m", p=128)
    epn_d = epn.rearrange("(j p) h -> p j h", p=128)
    w_ch, wt_ch, epT_ch, ep_ch = [], [], [], []
    _gate = _fact_dmas[15].ins
    _prev_small = None
    def _small_dma(dst, src):
        nonlocal _prev_small
        d = nc.scalar.dma_start(out=dst, in_=src)
        if _prev_small is None:
            tile.add_dep_helper(d.ins, _gate, sync=True, reason="factT first")
        else:
            tile.add_dep_helper(d.ins, _prev_small, sync=False, reason="ACT ring order")
        _prev_small = d.ins
        return d
    for c in range(HC):
        t = smw.tile([128, H], F32R, tag="w")
        _small_dma(t, w_d[:, c, :])
        w_ch.append(t)
    for c in range(HC):
        t = smept.tile([128, MS], F32R, tag="epT")
        _small_dma(t, epT_d[:, c, :])
        epT_ch.append(t)
    for jj in range(MQ):
        t = smep.tile([128, H], F32R, tag="ep")
        _small_dma(t, epn_d[:, jj, :])
        ep_ch.append(t)
    for c in range(HC):
        t = fst.tile([128, H], F32R, tag="fst")
        _small_dma(t, wt_d[:, c, :])
        wt_ch.append(t)
    # fnorm on the SP ring right after factT (needed only by the final
    # weighted sum; the SBUF-to-SBUF gathers don't contend with it)
    fnorm_sb = big.tile([128, NT, H], BF16, tag="fnorm")
    _fn_d = factn.rearrange("(j p) h -> p j h", p=128)
    nc.sync.dma_start(out=fnorm_sb[:, 0 : NT // 2, :], in_=_fn_d[:, 0 : NT // 2, :])
    (nc.gpsimd if dq else nc.sync).dma_start(
        out=fnorm_sb[:, NT // 2 : NT, :], in_=_fn_d[:, NT // 2 : NT, :])

    if stage < 1:
        _dummy_out(nc, rows, out_ep, out_st)
        return
    def _warm(n, c=0, off=0):
        for _ in range(n):
            ps_w = psE.tile([1, 512], F32, tag="psE")
            nc.tensor.matmul(ps_w, ones_colr, factT_sb[:, c, off : off + 512],
                             start=True, stop=True)
    for k in range(12):
        _warm(w1, c=k // 2, off=(k % 2) * 2048)
    # ---- pass 1 finish: combine quarter-chunk maxes into fQl [128, HC]
    fQl = tiny.tile([128, HC], F32, tag="fQl")
    r1 = nc.vector.reduce_max(fQl, fQh, axis=mybir.AxisListType.X)
    if rep > 0 and comm != "cc" and not sim1:
        pools["_waits"].append((r1, lsem1, LSEM_PER_REP * rep))
    _warm(w2, c=HC - 1, off=2048)

    if stage < 11:
        _dummy_out(nc, rows, out_ep, out_st)
        return
    # ---- AG1: all-gather the local column maxes, then reduce (max is
    # permutation-invariant across slots)
    par = rep % 2
    fQg = tiny.tile([128, HC], F32R, tag="fQg")
    if comm == "cc" and not sim1:
        dram = pool("dram", bufs=1, space="DRAM")
        cc1_in = dram.tile([H], F32, tag="cc1i")
        cc1_out = dram.tile([H], F32, tag="cc1o")
        ceng.dma_start(out=cc1_in[:].rearrange("(c p) -> p c", p=128), in_=fQl)
        nc.gpsimd.collective_compute(
            "AllReduce", mybir.AluOpType.max, replica_groups=RG,
            ins=[cc1_in.opt()], outs=[cc1_out.opt()],
        )
        ceng.dma_start(out=fQg, in_=cc1_out[:].rearrange("(c p) -> p c", p=128))
    else:
        Y1 = tiny.tile([128, 2, NCORES, HC], F32, tag="Y1")
        if sim1:
            for k in range(NCORES):
                nc.vector.tensor_copy(Y1[:, par, k, :], fQl)
        else:
            _allgather(nc, pools, fQl, Y1[:, par], rsem1, lsem1, rep, "ag1")
        m4 = tiny.tile([128, 4, HC], F32, tag="m4")
        m2 = tiny.tile([128, 2, HC], F32, tag="m2")
        x1 = nc.vector.tensor_tensor(out=m4, in0=Y1[:, par, 0:4, :], in1=Y1[:, par, 4:8, :],
                                     op=mybir.AluOpType.max)
        if not sim1:
            pools["_waits"].append((x1, rsem1, RSEM_PER_REP * (rep + 1)))
        nc.vector.tensor_tensor(out=m2, in0=m4[:, 0:2, :], in1=m4[:, 2:4, :],
                                op=mybir.AluOpType.max)
        nc.vector.tensor_tensor(out=fQg, in0=m2[:, 0, :], in1=m2[:, 1, :],
                                op=mybir.AluOpType.max)

    if stage < 12:
        _dummy_out(nc, rows, out_ep, out_st)
        return
    regions768 = [(0, 512), (512, 256)]

    # ---- qw = fact_Q @ W  (contraction over h; lhsT = fQg chunks)
    ps_qw = psA.tile([1, H], F32, tag="psA")
    for c in range(HC):
        for n0, nn in regions768:
            nc.tensor.matmul(
                ps_qw[:, n0 : n0 + nn], fQg[:, c : c + 1], w_ch[c][:, n0 : n0 + nn],
                start=(c == 0), stop=(c == HC - 1),
            )
    qw_row = rows.tile([1, H], F32, tag="rows")
    nc.scalar.activation(out=qw_row, in_=ps_qw, func=mybir.ActivationFunctionType.Copy)

    # transpose qw row -> [128, HC] via K=1 matmuls against ones
    qwT = tiny.tile([128, HC], F32R, tag="qwT")
    ps_t = psT.tile([128, HC], F32, tag="psT")
    for c in range(HC):
        nc.tensor.matmul(ps_t[:, c : c + 1], qw_row[:, c * 128 : (c + 1) * 128], ones11,
                         start=True, stop=True)
    nc.vector.tensor_copy(qwT, ps_t)

    if stage < 13:
        _dummy_out(nc, rows, out_ep, out_st)
        return
    # ---- f2e local logits [1, MS]
    ps_f2e = psA.tile([1, MS], F32, tag="psA")
    for c in range(HC):
        nc.tensor.matmul(
            ps_f2e, qwT[:, c : c + 1], epT_ch[c],
            start=(c == 0), stop=(c == HC - 1),
        )
    # local softmax pieces
    cmax_l = tiny.tile([1, 1], F32, tag="cmax_l")
    neg_cmax = tiny.tile([1, 1], F32, tag="neg_cmax")
    nc.vector.reduce_max(cmax_l, ps_f2e, axis=mybir.AxisListType.X)
    nc.vector.tensor_reduce(neg_cmax, ps_f2e, axis=mybir.AxisListType.X,
                            op=mybir.AluOpType.max, negate=True)
    p_loc = rows.tile([1, MS], F32, tag="rows")
    s_loc = tiny.tile([1, 1], F32, tag="s_loc")
    nc.scalar.activation(
        out=p_loc, in_=ps_f2e, func=mybir.ActivationFunctionType.Exp,
        bias=neg_cmax, scale=1.0, accum_out=s_loc,
    )

    # transpose p_loc [1,512] -> pT [128, MQ] (m = jj*128 + p)
    pT = tiny.tile([128, MQ], F32R, tag="pT")
    ps_t2 = psT.tile([128, MQ], F32, tag="psT")
    for jj in range(MQ):
        nc.tensor.matmul(ps_t2[:, jj : jj + 1], p_loc[:, jj * 128 : (jj + 1) * 128], ones11,
                         start=True, stop=True)
    nc.vector.tensor_copy(pT, ps_t2)

    if stage < 14:
        _dummy_out(nc, rows, out_ep, out_st)
        return
    # ---- accep_local = p_loc @ ep_local  [1, H]
    ps_accep = psA.tile([1, H], F32, tag="psA")
    for jj in range(MQ):
        for n0, nn in regions768:
            nc.tensor.matmul(
                ps_accep[:, n0 : n0 + nn], pT[:, jj : jj + 1], ep_ch[jj][:, n0 : n0 + nn],
                start=(jj == 0), stop=(jj == MQ - 1),
            )

    if stage < 15:
        r = rows.tile([1, H], F32, tag="rows")
        nc.vector.tensor_copy(r, ps_accep)
        nc.gpsimd.dma_start(out=out_ep, in_=r)
        d2 = rows.tile([1, H + 2], F32, tag="row770")
        nc.vector.memset(d2, 0.0)
        nc.gpsimd.dma_start(out=out_st, in_=d2)
        return
    # ---- AG2: pack [accep cols | cmax | s] as a [128, 8] column payload
    # (accep transposed to h-column form, stats on partition 0), gather,
    # and combine locally — identical result on every core.
    accep_row = rows.tile([1, H], F32, tag="rows")
    nc.scalar.activation(out=accep_row, in_=ps_accep,
                         func=mybir.ActivationFunctionType.Copy)
    ps_t5 = psT.tile([128, HC], F32, tag="psT")
    for c in range(HC):
        nc.tensor.matmul(ps_t5[:, c : c + 1], accep_row[:, c * 128 : (c + 1) * 128],
                         ones11, start=True, stop=True)
    pay = tiny.tile([128, NCORES], F32, tag="pay")
    pw = nc.vector.tensor_copy(pay[:, 0:HC], ps_t5)
    if rep > 0 and comm != "cc" and not sim1:
        pools["_waits"].append((pw, lsem2, LSEM_PER_REP * rep))
    # chain the other pay writers behind pw so the lsem2 drain wait
    # covers every overwrite of the in-flight send buffer
    ms = nc.vector.memset(pay[:, HC : HC + 2], 0.0)
    tile.add_dep_helper(ms.ins, pw.ins, sync=False, reason="pay WAR order")
    for wi in (
        nc.vector.tensor_copy(pay[0:1, HC : HC + 1], cmax_l),
        nc.vector.tensor_copy(pay[0:1, HC + 1 : HC + 2], s_loc),
    ):
        tile.add_dep_helper(wi.ins, ms.ins, sync=False, reason="stats after memset")
    Y2 = tiny.tile([128, 2, NCORES, NCORES], F32, tag="Y2")
    if comm == "cc" and not sim1:
        dram = pool("dram", bufs=1, space="DRAM")
        cc2_in = dram.tile([128 * NCORES], F32, tag="cc2i")
        cc2_out = dram.tile([NCORES * 128 * NCORES], F32, tag="cc2o")
        ceng.dma_start(out=cc2_in[:].rearrange("(p c) -> p c", p=128), in_=pay)
        nc.gpsimd.collective_compute(
            "AllGather", mybir.AluOpType.bypass, replica_groups=RG,
            ins=[cc2_in.opt()], outs=[cc2_out.opt()],
        )
        ceng.dma_start(
            out=Y2[:, par],
            in_=cc2_out[:].rearrange("(r p c) -> p r c", p=128, c=NCORES))
    elif sim1:
        for k in range(NCORES):
            nc.vector.tensor_copy(Y2[:, par, k, :], pay)
    else:
        _allgather(nc, pools, pay, Y2[:, par], rsem2, lsem2, rep, "ag2")
    _warm(w3, c=0, off=0)

    # combine: coef_r = exp(cmax_r - max_r cmax_r); D = sum coef_r * s_r;
    # ep_col = sum (coef_r / D) * accep_col_r   (slot-order invariant)
    cmrow = tiny.tile([1, NCORES], F32, tag="cmrow")
    x2w = nc.vector.tensor_copy(
        cmrow, Y2[0:1, par, :, HC : HC + 1].rearrange("p r one -> p (r one)"))
    if not sim1 and comm != "cc":
        pools["_waits"].append((x2w, rsem2, RSEM_PER_REP * (rep + 1)))
    srow = tiny.tile([1, NCORES], F32, tag="srow")
    sr = nc.vector.tensor_copy(
        srow, Y2[0:1, par, :, HC + 1 : HC + 2].rearrange("p r one -> p (r one)"))
    tile.add_dep_helper(sr.ins, x2w.ins, sync=False, reason="after AG2 arrival wait")
    negc2 = tiny.tile([1, 1], F32, tag="negc2")
    nc.vector.tensor_reduce(negc2, cmrow, axis=mybir.AxisListType.X,
                            op=mybir.AluOpType.max, negate=True)
    coef = tiny.tile([1, NCORES], F32, tag="coef")
    nc.scalar.activation(out=coef, in_=cmrow, func=mybir.ActivationFunctionType.Exp,
                         bias=negc2, scale=1.0)
    prod = tiny.tile([1, NCORES], F32, tag="prod")
    nc.vector.tensor_tensor(out=prod, in0=coef, in1=srow,
                            op=mybir.AluOpType.mult)
    D = tiny.tile([1, 1], F32, tag="D")
    nc.vector.reduce_sum(D, prod, axis=mybir.AxisListType.X)
    Sinv = tiny.tile([1, 1], F32, tag="Sinv")
    nc.vector.reciprocal(Sinv, D)
    csc = tiny.tile([1, NCORES], F32, tag="csc")
    nc.vector.tensor_scalar_mul(csc, coef, Sinv)
    # broadcast csc across partitions via ones-matmul
    ps_cb = psT.tile([128, NCORES], F32, tag="psT2")
    nc.tensor.matmul(ps_cb, ones_col[0:1, :].broadcast_to([1, 128]), csc,
                     start=True, stop=True)
    cscB = tiny.tile([128, NCORES], F32, tag="cscB")
    nc.vector.tensor_copy(cscB, ps_cb)
    # ep_col = sum_r cscB[:, r] * accep_col_r
    sc = tiny.tile([128, NCORES, HC], F32, tag="sc")
    for r in range(NCORES):
        nc.vector.tensor_scalar_mul(sc[:, r, :], Y2[:, par, r, 0:HC], cscB[:, r : r + 1])
    sa = tiny.tile([128, 4, HC], F32, tag="sa")
    sb = tiny.tile([128, 2, HC], F32, tag="sb")
    ep_col = tiny.tile([128, HC], F32, tag="ep_col")
    nc.vector.tensor_tensor(out=sa, in0=sc[:, 0:4, :], in1=sc[:, 4:8, :],
                            op=mybir.AluOpType.add)
    nc.vector.tensor_tensor(out=sb, in0=sa[:, 0:2, :], in1=sa[:, 2:4, :],
                            op=mybir.AluOpType.add)
    nc.vector.tensor_tensor(out=ep_col, in0=sb[:, 0, :], in1=sb[:, 1, :],
                            op=mybir.AluOpType.add)
    ceng.dma_start(out=out_ep.rearrange("a (c p) -> p (a c)", p=128), in_=ep_col)
    ep_colr = tiny.tile([128, HC], F32R, tag="ep_colr")
    nc.vector.tensor_copy(ep_colr, ep_col)

    # ---- v = W @ ep_.T  as a row: v_row = ep_ @ W.T  [1, H]
    ps_v = psA.tile([1, H], F32, tag="psA")
    for c in range(HC):
        for n0, nn in regions768:
            nc.tensor.matmul(
                ps_v[:, n0 : n0 + nn], ep_colr[:, c : c + 1], wt_ch[c][:, n0 : n0 + nn],
                start=(c == 0), stop=(c == HC - 1),
            )
    v_row = rows.tile([1, H], F32, tag="rows")
    nc.scalar.activation(out=v_row, in_=ps_v, func=mybir.ActivationFunctionType.Copy)
    vT = tiny.tile([128, HC], fdt, tag="vT")
    ps_t4 = psT.tile([128, HC], F32, tag="psT")
    for c in range(HC):
        nc.tensor.matmul(ps_t4[:, c : c + 1], v_row[:, c * 128 : (c + 1) * 128], ones11,
                         start=True, stop=True)
    nc.vector.tensor_copy(vT, ps_t4)

    if stage < 16:
        d2 = rows.tile([1, H + 2], F32, tag="row770")
        nc.vector.memset(d2, 0.0)
        nc.gpsimd.dma_start(out=out_st, in_=d2)
        return
    # ---- e2f local logits [1, NS] in float32r, transposed to [128, NT]
    # via K=1 matmuls (n = j*128 + p)
    x2 = psT.tile([128, NT], F32, tag="psX")
    _pend = None
    for b in range(NB):
        ps_e2f = psE.tile([1, 512], F32, tag="psE")
        for c in range(HC):
            nc.tensor.matmul(
                ps_e2f,
                vT[:, c : c + 1],
                factT_sb[:, c, b * 512 : (b + 1) * 512],
                start=(c == 0), stop=(c == HC - 1),
            )
        if _pend is not None:
            pb, prow = _pend
            for jj in range(4):
                j = pb * 4 + jj
                nc.tensor.matmul(x2[:, j : j + 1], prow[:, jj * 128 : (jj + 1) * 128],
                                 ones11, start=True, stop=True)
        e2f_row = rows.tile([1, 512], F32, tag="rows")
        nc.scalar.activation(out=e2f_row, in_=ps_e2f,
                             func=mybir.ActivationFunctionType.Copy)
        _pend = (b, e2f_row)
    pb, prow = _pend
    for jj in range(4):
        j = pb * 4 + jj
        nc.tensor.matmul(x2[:, j : j + 1], prow[:, jj * 128 : (jj + 1) * 128],
                         ones11, start=True, stop=True)

    # ---- e2f softmax partials (n = p*NT + j)
    m_p = tiny.tile([128, 1], F32, tag="m_p")
    nc.vector.reduce_max(m_p, x2, axis=mybir.AxisListType.X)
    ps_mr = psT.tile([1, 128], F32, tag="psT")
    nc.tensor.matmul(ps_mr, m_p, ident, start=True, stop=True)
    Cg = tiny.tile([1, 1], F32, tag="Cg")
    negCg = tiny.tile([1, 1], F32, tag="negCg")
    nc.vector.reduce_max(Cg, ps_mr, axis=mybir.AxisListType.X)
    nc.vector.tensor_reduce(negCg, ps_mr, axis=mybir.AxisListType.X,
                            op=mybir.AluOpType.max, negate=True)
    ps_cb2 = psT.tile([128, 1], F32, tag="psT2")
    nc.tensor.matmul(ps_cb2, ones_col[0:1, :].broadcast_to([1, 128]), negCg,
                     start=True, stop=True)
    negC = tiny.tile([128, 1], F32, tag="negC")
    nc.vector.tensor_copy(negC, ps_cb2)
    s_ps = tiny.tile([128, 1], F32, tag="s_ps")
    p2bf = tiny.tile([128, NT], BF16, tag="p2bf")
    nc.scalar.activation(
        out=p2bf, in_=x2, func=mybir.ActivationFunctionType.Exp,
        bias=negC, scale=1.0, accum_out=s_ps,
    )
    ps_S = psT.tile([1, 1], F32, tag="psT3")
    nc.tensor.matmul(ps_S, s_ps, ones_col, start=True, stop=True)

    if stage < 17:
        d2 = rows.tile([1, H + 2], F32, tag="row770")
        nc.vector.memset(d2, 0.0)
        nc.gpsimd.dma_start(out=out_st, in_=d2)
        return
    # ---- acc = sum_n p_n * fact[n, :]  (bf16 matmuls, fp32 accumulate)
    ps_acc = psA.tile([1, H], F32, tag="psA")
    for n0, nn in regions768:
        for j in range(NT):
            nc.tensor.matmul(
                ps_acc[:, n0 : n0 + nn], p2bf[:, j : j + 1], fnorm_sb[:, j, n0 : n0 + nn],
                start=(j == 0), stop=(j == NT - 1),
            )

    # ---- stats out: [C, S, acc]
    st_row = rows.tile([1, H + 2], F32, tag="row770")
    nc.vector.tensor_copy(st_row[:, 0:1], Cg)
    nc.vector.tensor_copy(st_row[:, 1:2], ps_S)
    nc.scalar.activation(out=st_row[:, 2 : H + 2], in_=ps_acc,
                         func=mybir.ActivationFunctionType.Copy)
    ceng.dma_start(out=out_st, in_=st_row)


def build(stage=999, reps=1, sim1=False, comm=None, dq=None, fdt=None, warm=None, ccq=None):
    fdt = {None: FDT, "f32r": "f32r", "bf16": "bf16"}[fdt]
    fdt = {"f32r": F32R, "bf16": BF16}[fdt]
    nc = bacc.Bacc("TRN2", target_bir_lowering=False, debug=False,
                   num_devices=1 if sim1 else NCORES)
    factT = nc.dram_tensor("factTb" if fdt == BF16 else "factT",
                           [H, NS], fdt, kind="ExternalInput").ap()
    factn = nc.dram_tensor("factn", [NS, H], BF16, kind="ExternalInput").ap()
    epT = nc.dram_tensor("epT", [H, MS], F32R, kind="ExternalInput").ap()
    epn = nc.dram_tensor("epn", [MS, H], F32R, kind="ExternalInput").ap()
    w = nc.dram_tensor("w", [H, H], F32R, kind="ExternalInput").ap()
    wt = nc.dram_tensor("wt", [H, H], F32R, kind="ExternalInput").ap()
    out_ep = nc.dram_tensor("out_ep", [1, H], F32, kind="ExternalOutput").ap()
    out_st = nc.dram_tensor("out_st", [1, H + 2], F32, kind="ExternalOutput").ap()
    from contextlib import ExitStack

    pools = {}
    with tile.TileContext(nc) as tc:
        with ExitStack() as ctx:
            for _ in range(reps):
                _body(nc, tc, ctx, factT, factn, epT, epn, w, wt, out_ep, out_st,
                      stage=stage, pools=pools, sim1=sim1, comm=comm, dq=dq, fdt=fdt,
                      warm=warm, ccq=ccq)
    # attach remote-arrival / drain waits after tile scheduling (the
    # scheduler's single-core sim cannot satisfy remote increments);
    # compile()'s generate_event_semaphores legalizes multi-wait insts
    for inst, sem, thr in pools.get("_waits", []):
        inst.wait_op(sem, thr, "sem-ge", check=False)
    nc.compile()
    return nc


def make_in_maps(fact, elements_p, weights):
    fact = np.asarray(fact, np.float32)
    elements_p = np.asarray(elements_p, np.float32)
    weights = np.asarray(weights, np.float32)
    wt = np.ascontiguousarray(weights.T)
    in_maps = []
    for i in range(NCORES):
        fs = fact[i * NS : (i + 1) * NS]
        es = elements_p[i * MS : (i + 1) * MS]
        fsT = np.ascontiguousarray(fs.T)
        in_maps.append({
            "factT": fsT,
            **({"factTb": fsT.astype(ml_dtypes.bfloat16)} if FDT == "bf16" else {}),
            "factn": fs.astype(ml_dtypes.bfloat16),
            "epT": np.ascontiguousarray(es.T),
            "epn": np.ascontiguousarray(es),
            "w": weights,
            "wt": wt,
        })
    return in_maps


def combine(results):
    st = np.stack([np.asarray(results[i]["out_st"][0], np.float64) for i in range(NCORES)])
    C = st[:, 0]
    S = st[:, 1]
    acc = st[:, 2:]
    Cg = C.max()
    coef = np.exp(C - Cg)
    fact_ = (coef[:, None] * acc).sum(0, keepdims=True) / (coef * S).sum()
    ep_ = np.asarray(results[0]["out_ep"], np.float32)
    return fact_.astype(np.float32), ep_


def kernel(fact, elements_p, weights, **run_kwargs):
    if "nc" not in _CACHE:
        _CACHE["nc"] = build()
    nc = _CACHE["nc"]
    in_maps = make_in_maps(fact, elements_p, weights)
    res = bass_utils.run_bass_kernel_spmd(
        nc, in_maps, core_ids=list(range(NCORES)), **run_kwargs
    )
    _CACHE["last_result"] = res
    return combine(res.results)


def build_sim1():
    return build(sim1=True)



# revision 31
# speedup vs baseline: 1.6080x; 1.2608x over previous
"""Trainium2 Bass kernel for nn_AggregationLayer (pooling).

Computation (reference):
    fact: [N,H], elements_p: [M,H], weights: [H,H]   (N=32768, M=4096, H=768)
    fact_Q = max(fact, axis=0)                     # [1,H]
    f2e = softmax(fact_Q @ W @ ep.T)               # [1,M]
    ep_ = f2e @ ep                                 # [1,H]
    e2f = softmax_n(fact @ W @ ep_.T)              # [N,1]
    fact_ = e2f.T @ fact                           # [1,H]
    returns (fact_, ep_)

Distribution over 8 NeuronCores:
    fact sharded along N (4096 rows per core, sent in 2 layouts:
    float32r transposed [H, N/8] DMA'd straight into the resident PE
    matvec tile — f32r storage is plain fp32 bytes, giving full-rate
    PE matmuls while the col-max reduce stays fp32-exact — and bf16
    row-major for the final weighted row-sum), elements_p sharded
    along M (512 rows per core, both layouts f32r), weights replicated
    f32r (W and W.T). Cross-core: one AllReduce(max) for fact_Q and
    one AllGather of a [128, 8] column-form payload (accep transposed
    to h-columns + the two softmax stats), combined on every core with
    slot-order-invariant math. A remote_dma_broadcast SBUF-to-SBUF
    all-gather path exists behind COMM="rdma" (correct on HW, but the
    per-broadcast Q7 SWDGE descriptor generation costs more per rep
    than ncfw collectives). The final e2f softmax partials (one scalar
    max, one scalar sum, one [768] weighted sum per core) are combined
    on host.

Precision envelope (measured end-to-end on the seed data, tol 2e-2):
    The f2e softmax logits are ~234 in magnitude with a top-2 gap of
    1.78, and p1 = [0.856, 0.144, ~0] — NOT one-hot.  Any bf16
    rounding upstream of those logits (fact col-max, W, or ep) shifts
    the logits by O(0.3-0.9) and blows fact_ to 4e-2..1.4e-1.  So
    fact/W/ep must stay fp32-stored (f32r matmul mode is fine); only
    the final weighted row-sum source (fnorm) tolerates bf16
    (1.8e-3).  fdt="bf16" exists but fails tolerance — do not ship.

Timing methodology (axon): each RPC dispatch costs ~90 ms with ~2 ms
    jitter while 32 extra in-NEFF reps add only ~1.5-4 ms, so per-rep
    time needs hundreds of alternated samples and trimmed means (see
    test.py).  Absolute per-rep throughput drifts 2-3x over tens of
    minutes on this shared device; only interleaved A/B comparisons
    (ab.py) are decision-grade.  Null results (all within +/-8 us of
    zero, 300-round interleaved A/B): warm=(3,10,8) vs (3,25,18);
    dq="tri" (3-way factT load split adding the ACT ring) combined
    with ccq="act" (collective staging + output DMAs on HWDGE instead
    of gpsimd SWDGE).  Replicating ep to remove the AllGather is dead
    on SBUF capacity (needs +22 MB fp32; fp32 is mandatory — see
    precision envelope above).
"""
import sys

sys.path.insert(0, "/opt/trn_rl_repo")

import numpy as np
import ml_dtypes

import concourse.bass as bass
import concourse.tile as tile
import concourse.mybir as mybir
from concourse import bacc, bass_utils

N, M, H = 32768, 4096, 768
NCORES = 8
NS = N // NCORES          # 4096 fact rows per core
MS = M // NCORES          # 512 ep rows per core
HC = H // 128             # 6 h-chunks of 128
NB = NS // 512            # 8 psum banks for e2f
NT = NS // 128            # 32 acc tiles
MQ = MS // 128            # 4 accep tiles

F32 = mybir.dt.float32
F32R = mybir.dt.float32r
BF16 = mybir.dt.bfloat16
RG = [list(range(NCORES))]

# PE clock-keeper matmul counts: W1 groups during the factT load,
# W2 during the AG1 (col-max exchange) flight, W3 during AG2.
W1, W2, W3 = 3, 25, 18

# cross-core exchange implementation: "rdma" = remote_dma_broadcast
# all-gathers, "cc" = ncfw collective_compute (single AllReduce + AllGather).
# Measured on HW (interleaved A/B, 33-rep NEFFs): cc 157.5us/rep,
# baseline 183.7us/rep, rdma 232.6us/rep (Q7 SWDGE desc-gen per
# broadcast dominates) -> cc.
COMM = "cc"

# factT storage/matvec dtype: "f32r" (exact col-max, f32r matvec) or
# "bf16" (half the dominant DMA; col-max+matvec in bf16).
import os
FDT = os.environ.get("KFDT", "f32r")

# split the big fact loads across two HWDGE queues (SP + Pool):
# measured -13us/rep vs single-queue in an interleaved A/B (per-queue
# descriptor throughput, not HBM, was the limiter), identical numerics
DQ = True

# per-broadcast semaphore increments: local_sem +16 per broadcast,
# remote_sem +2 (16 lanes / 8 slots) per delivery; 7 peers per gather
LSEM_PER_REP = 16 * (NCORES - 1)
RSEM_PER_REP = 2 * (NCORES - 1)

_CACHE = {}


def _dummy_out(nc, rows, out_ep, out_st):
    d1 = rows.tile([1, H], F32, tag="rows")
    d2 = rows.tile([1, H + 2], F32, tag="row770")
    nc.vector.memset(d1, 0.0)
    nc.vector.memset(d2, 0.0)
    nc.gpsimd.dma_start(out=out_ep, in_=d1)
    nc.gpsimd.dma_start(out=out_st, in_=d2)


def _allgather(nc, pools, src, dst_par, rsem, lsem, rep, tag):
    """All-gather src [128, W] into dst_par [128, 8, W] (slot 0 = self,
    slot k = phys-peer XOR k). Returns the trigger instruction. Caller
    attaches the arrival wait (rsem >= RSEM_PER_REP*(rep+1)) to the
    first consumer of dst_par."""
    c0 = nc.vector.tensor_copy(dst_par[:, 0, :], src)
    first = None
    for k in range(1, NCORES):
        rdests = [None] * NCORES
        rdests[k] = (0, k)
        p = nc.gpsimd.remote_dma_broadcast(
            out_ap=dst_par[:, k, :], in_ap=src,
            remote_sem=rsem, local_sem=lsem, rdests=rdests,
        )
        if first is None:
            first = p
    if rep == 0:
        # all-cores-entered barrier before the first remote send of the
        # kernel (prelude AllGather inc; overlaps the input load)
        nc._bir_kernel_barrier_sem_replica_groups.extend(set(g) for g in RG)
        pools["_waits"].append(
            (first, nc._bir_kernel_barrier_sem, nc.bir_kernel_barrier_sem_inc))
    nc.gpsimd.trigger_dma(count=None)
    return c0


def _body(nc, tc, ctx, factT, factn, epT, epn, w, wt, out_ep, out_st, stage=999, pools=None, sim1=False, comm=None, dq=None, fdt=F32R, warm=None, ccq=None):
    w1, w2, w3 = warm if warm is not None else (W1, W2, W3)
    # ccq='act': stage collective payloads and kernel outputs through the
    # ACT HWDGE ring instead of gpsimd SWDGE (skips per-DMA descriptor
    # generation on the Q7 ucode path).
    ceng = nc.scalar if ccq == "act" else nc.gpsimd
    if pools is None:
        pools = {}
    if comm is None:
        comm = COMM
    if dq is None:
        dq = DQ
    rep = pools.setdefault("_rep", 0)
    pools["_rep"] = rep + 1
    pools.setdefault("_waits", [])
    def pool(name, **kw):
        if name not in pools:
            pools[name] = ctx.enter_context(tc.tile_pool(name=name, **kw))
        return pools[name]
    big = pool("big", bufs=1)
    rows = pool("rows", bufs=2)
    tiny = pool("tiny", bufs=1)
    psA = pool("psA", bufs=1, space="PSUM")
    psE = pool("psE", bufs=2, space="PSUM")
    psT = pool("psT", bufs=1, space="PSUM")
    if "consts" not in pools:
        ones11_t = tiny.tile([1, 1], F32, tag="ones11")
        nc.vector.memset(ones11_t, 1.0)
        ones_col_t = tiny.tile([128, 1], F32, tag="ones_col")
        nc.vector.memset(ones_col_t, 1.0)
        ident_d = nc.inline_tensor(np.eye(128, dtype=np.float32), name="ident128")
        ident_t = tiny.tile([128, 128], F32, tag="ident128")
        nc.sync.dma_start(out=ident_t, in_=ident_d.ap())
        onesr_t = tiny.tile([128, 1], fdt, tag="ones_colr")
        nc.vector.tensor_copy(onesr_t, ones_col_t)
        sems = (nc.alloc_semaphore("ag1_r"), nc.alloc_semaphore("ag1_l"),
                nc.alloc_semaphore("ag2_r"), nc.alloc_semaphore("ag2_l"))
        pools["consts"] = (ones11_t, ones_col_t, ident_t, onesr_t)
        pools["sems"] = sems
    ones11, ones_col, ident, ones_colr = pools["consts"]
    rsem1, lsem1, rsem2, lsem2 = pools["sems"]

    # ---- big input loads. factT streams in as fp32-bit-exact float32r
    # quarter-chunks on the SP HWDGE ring straight into the resident PE
    # matvec tile (f32r storage is plain fp32 bytes; the rounding is a PE
    # matmul mode), and each quarter is reduced in place for the exact
    # column max. fnorm follows on the same ring.
    QRT = NS // 4
    fst = pool("fst", bufs=6)  # holds the 6 resident wt chunks
    factT_sb = big.tile([128, HC, NS], fdt, tag="factT")
    factT_d = factT.rearrange("(c p) (u n) -> p c u n", p=128, u=4)
    fQh = tiny.tile([128, HC, 4], F32, tag="fQh")
    _fact_dmas = []
    engs3 = [nc.sync, nc.gpsimd, nc.scalar]
    for c in range(HC):
        for u in range(4):
            sl = factT_sb[:, c, u * QRT : (u + 1) * QRT]
            i = c * 4 + u
            if dq == "tri":
                # 3-way split of the critical factT load: SP + SWDGE +
                # ACT rings (small loads queue behind ACT's share via
                # ring FIFO + the explicit gate below)
                eng = engs3[i % 3]
            else:
                eng = nc.gpsimd if (dq and i % 2) else nc.sync
            _fact_dmas.append(eng.dma_start(out=sl, in_=factT_d[:, c, u, :]))
            nc.vector.reduce_max(fQh[:, c, u : u + 1], sl, axis=mybir.AxisListType.X)

    # ---- small input loads (ACT HWDGE ring, parallel with the SP ring)
    smw = pool("smw", bufs=2)
    smept = pool("smept", bufs=2)
    smep = pool("smep", bufs=2)
    w_d = w.rearrange("(c p) j -> p c j", p=128)
    wt_d = wt.rearrange("(c p) j -> p c j", p=128)
    epT_d = epT.rearrange("(c p) m -> p c m", p=128)
    epn_d = epn.rearrange("(j p) h -> p j h", p=128)
    w_ch, wt_ch, epT_ch, ep_ch = [], [], [], []
    _gate = _fact_dmas[15].ins
    _prev_small = None
    def _small_dma(dst, src):
        nonlocal _prev_small
        d = nc.scalar.dma_start(out=dst, in_=src)
        if _prev_small is None:
            tile.add_dep_helper(d.ins, _gate, sync=True, reason="factT first")
        else:
            tile.add_dep_helper(d.ins, _prev_small, sync=False, reason="ACT ring order")
        _prev_small = d.ins
        return d
    for c in range(HC):
        t = smw.tile([128, H], F32R, tag="w")
        _small_dma(t, w_d[:, c, :])
        w_ch.append(t)
    for c in range(HC):
        t = smept.tile([128, MS], F32R, tag="epT")
        _small_dma(t, epT_d[:, c, :])
        epT_ch.append(t)
    for jj in range(MQ):
        t = smep.tile([128, H], F32R, tag="ep")
        _small_dma(t, epn_d[:, jj, :])
        ep_ch.append(t)
    for c in range(HC):
        t = fst.tile([128, H], F32R, tag="fst")
        _small_dma(t, wt_d[:, c, :])
        wt_ch.append(t)
    # fnorm on the SP ring right after factT (needed only by the final
    # weighted sum; the SBUF-to-SBUF gathers don't contend with it)
    fnorm_sb = big.tile([128, NT, H], BF16, tag="fnorm")
    _fn_d = factn.rearrange("(j p) h -> p j h", p=128)
    nc.sync.dma_start(out=fnorm_sb[:, 0 : NT // 2, :], in_=_fn_d[:, 0 : NT // 2, :])
    (nc.gpsimd if dq else nc.sync).dma_start(
        out=fnorm_sb[:, NT // 2 : NT, :], in_=_fn_d[:, NT // 2 : NT, :])

    if stage < 1:
        _dummy_out(nc, rows, out_ep, out_st)
        return
    def _warm(n, c=0, off=0):
        for _ in range(n):
            ps_w = psE.tile([1, 512], F32, tag="psE")
            nc.tensor.matmul(ps_w, ones_colr, factT_sb[:, c, off : off + 512],
                             start=True, stop=True)
    for k in range(12):
        _warm(w1, c=k // 2, off=(k % 2) * 2048)
    # ---- pass 1 finish: combine quarter-chunk maxes into fQl [128, HC]
    fQl = tiny.tile([128, HC], F32, tag="fQl")
    r1 = nc.vector.reduce_max(fQl, fQh, axis=mybir.AxisListType.X)
    if rep > 0 and comm != "cc" and not sim1:
        pools["_waits"].append((r1, lsem1, LSEM_PER_REP * rep))
    _warm(w2, c=HC - 1, off=2048)

    if stage < 11:
        _dummy_out(nc, rows, out_ep, out_st)
        return
    # ---- AG1: all-gather the local column maxes, then reduce (max is
    # permutation-invariant across slots)
    par = rep % 2
    fQg = tiny.tile([128, HC], F32R, tag="fQg")
    if comm == "cc" and not sim1:
        dram = pool("dram", bufs=1, space="DRAM")
        cc1_in = dram.tile([H], F32, tag="cc1i")
        cc1_out = dram.tile([H], F32, tag="cc1o")
        ceng.dma_start(out=cc1_in[:].rearrange("(c p) -> p c", p=128), in_=fQl)
        nc.gpsimd.collective_compute(
            "AllReduce", mybir.AluOpType.max, replica_groups=RG,
            ins=[cc1_in.opt()], outs=[cc1_out.opt()],
        )
        # f32 -> f32r cast DMA: only SWDGE (gpsimd) can cast
        nc.gpsimd.dma_start(out=fQg, in_=cc1_out[:].rearrange("(c p) -> p c", p=128))
    else:
        Y1 = tiny.tile([128, 2, NCORES, HC], F32, tag="Y1")
        if sim1:
            for k in range(NCORES):
                nc.vector.tensor_copy(Y1[:, par, k, :], fQl)
        else:
            _allgather(nc, pools, fQl, Y1[:, par], rsem1, lsem1, rep, "ag1")
        m4 = tiny.tile([128, 4, HC], F32, tag="m4")
        m2 = tiny.tile([128, 2, HC], F32, tag="m2")
        x1 = nc.vector.tensor_tensor(out=m4, in0=Y1[:, par, 0:4, :], in1=Y1[:, par, 4:8, :],
                                     op=mybir.AluOpType.max)
        if not sim1:
            pools["_waits"].append((x1, rsem1, RSEM_PER_REP * (rep + 1)))
        nc.vector.tensor_tensor(out=m2, in0=m4[:, 0:2, :], in1=m4[:, 2:4, :],
                                op=mybir.AluOpType.max)
        nc.vector.tensor_tensor(out=fQg, in0=m2[:, 0, :], in1=m2[:, 1, :],
                                op=mybir.AluOpType.max)

    if stage < 12:
        _dummy_out(nc, rows, out_ep, out_st)
        return
    regions768 = [(0, 512), (512, 256)]

    # ---- qw = fact_Q @ W  (contraction over h; lhsT = fQg chunks)
    ps_qw = psA.tile([1, H], F32, tag="psA")
    for c in range(HC):
        for n0, nn in regions768:
            nc.tensor.matmul(
                ps_qw[:, n0 : n0 + nn], fQg[:, c : c + 1], w_ch[c][:, n0 : n0 + nn],
                start=(c == 0), stop=(c == HC - 1),
            )
    qw_row = rows.tile([1, H], F32, tag="rows")
    nc.scalar.activation(out=qw_row, in_=ps_qw, func=mybir.ActivationFunctionType.Copy)

    # transpose qw row -> [128, HC] via K=1 matmuls against ones
    qwT = tiny.tile([128, HC], F32R, tag="qwT")
    ps_t = psT.tile([128, HC], F32, tag="psT")
    for c in range(HC):
        nc.tensor.matmul(ps_t[:, c : c + 1], qw_row[:, c * 128 : (c + 1) * 128], ones11,
                         start=True, stop=True)
    nc.vector.tensor_copy(qwT, ps_t)

    if stage < 13:
        _dummy_out(nc, rows, out_ep, out_st)
        return
    # ---- f2e local logits [1, MS]
    ps_f2e = psA.tile([1, MS], F32, tag="psA")
    for c in range(HC):
        nc.tensor.matmul(
            ps_f2e, qwT[:, c : c + 1], epT_ch[c],
            start=(c == 0), stop=(c == HC - 1),
        )
    # local softmax pieces
    cmax_l = tiny.tile([1, 1], F32, tag="cmax_l")
    neg_cmax = tiny.tile([1, 1], F32, tag="neg_cmax")
    nc.vector.reduce_max(cmax_l, ps_f2e, axis=mybir.AxisListType.X)
    nc.vector.tensor_reduce(neg_cmax, ps_f2e, axis=mybir.AxisListType.X,
                            op=mybir.AluOpType.max, negate=True)
    p_loc = rows.tile([1, MS], F32, tag="rows")
    s_loc = tiny.tile([1, 1], F32, tag="s_loc")
    nc.scalar.activation(
        out=p_loc, in_=ps_f2e, func=mybir.ActivationFunctionType.Exp,
        bias=neg_cmax, scale=1.0, accum_out=s_loc,
    )

    # transpose p_loc [1,512] -> pT [128, MQ] (m = jj*128 + p)
    pT = tiny.tile([128, MQ], F32R, tag="pT")
    ps_t2 = psT.tile([128, MQ], F32, tag="psT")
    for jj in range(MQ):
        nc.tensor.matmul(ps_t2[:, jj : jj + 1], p_loc[:, jj * 128 : (jj + 1) * 128], ones11,
                         start=True, stop=True)
    nc.vector.tensor_copy(pT, ps_t2)

    if stage < 14:
        _dummy_out(nc, rows, out_ep, out_st)
        return
    # ---- accep_local = p_loc @ ep_local  [1, H]
    ps_accep = psA.tile([1, H], F32, tag="psA")
    for jj in range(MQ):
        for n0, nn in regions768:
            nc.tensor.matmul(
                ps_accep[:, n0 : n0 + nn], pT[:, jj : jj + 1], ep_ch[jj][:, n0 : n0 + nn],
                start=(jj == 0), stop=(jj == MQ - 1),
            )

    if stage < 15:
        r = rows.tile([1, H], F32, tag="rows")
        nc.vector.tensor_copy(r, ps_accep)
        nc.gpsimd.dma_start(out=out_ep, in_=r)
        d2 = rows.tile([1, H + 2], F32, tag="row770")
        nc.vector.memset(d2, 0.0)
        nc.gpsimd.dma_start(out=out_st, in_=d2)
        return
    # ---- AG2: pack [accep cols | cmax | s] as a [128, 8] column payload
    # (accep transposed to h-column form, stats on partition 0), gather,
    # and combine locally — identical result on every core.
    accep_row = rows.tile([1, H], F32, tag="rows")
    nc.scalar.activation(out=accep_row, in_=ps_accep,
                         func=mybir.ActivationFunctionType.Copy)
    ps_t5 = psT.tile([128, HC], F32, tag="psT")
    for c in range(HC):
        nc.tensor.matmul(ps_t5[:, c : c + 1], accep_row[:, c * 128 : (c + 1) * 128],
                         ones11, start=True, stop=True)
    pay = tiny.tile([128, NCORES], F32, tag="pay")
    pw = nc.vector.tensor_copy(pay[:, 0:HC], ps_t5)
    if rep > 0 and comm != "cc" and not sim1:
        pools["_waits"].append((pw, lsem2, LSEM_PER_REP * rep))
    # chain the other pay writers behind pw so the lsem2 drain wait
    # covers every overwrite of the in-flight send buffer
    ms = nc.vector.memset(pay[:, HC : HC + 2], 0.0)
    tile.add_dep_helper(ms.ins, pw.ins, sync=False, reason="pay WAR order")
    for wi in (
        nc.vector.tensor_copy(pay[0:1, HC : HC + 1], cmax_l),
        nc.vector.tensor_copy(pay[0:1, HC + 1 : HC + 2], s_loc),
    ):
        tile.add_dep_helper(wi.ins, ms.ins, sync=False, reason="stats after memset")
    Y2 = tiny.tile([128, 2, NCORES, NCORES], F32, tag="Y2")
    if comm == "cc" and not sim1:
        dram = pool("dram", bufs=1, space="DRAM")
        cc2_in = dram.tile([128 * NCORES], F32, tag="cc2i")
        cc2_out = dram.tile([NCORES * 128 * NCORES], F32, tag="cc2o")
        ceng.dma_start(out=cc2_in[:].rearrange("(p c) -> p c", p=128), in_=pay)
        nc.gpsimd.collective_compute(
            "AllGather", mybir.AluOpType.bypass, replica_groups=RG,
            ins=[cc2_in.opt()], outs=[cc2_out.opt()],
        )
        ceng.dma_start(
            out=Y2[:, par],
            in_=cc2_out[:].rearrange("(r p c) -> p r c", p=128, c=NCORES))
    elif sim1:
        for k in range(NCORES):
            nc.vector.tensor_copy(Y2[:, par, k, :], pay)
    else:
        _allgather(nc, pools, pay, Y2[:, par], rsem2, lsem2, rep, "ag2")
    _warm(w3, c=0, off=0)

    # combine: coef_r = exp(cmax_r - max_r cmax_r); D = sum coef_r * s_r;
    # ep_col = sum (coef_r / D) * accep_col_r   (slot-order invariant)
    cmrow = tiny.tile([1, NCORES], F32, tag="cmrow")
    x2w = nc.vector.tensor_copy(
        cmrow, Y2[0:1, par, :, HC : HC + 1].rearrange("p r one -> p (r one)"))
    if not sim1 and comm != "cc":
        pools["_waits"].append((x2w, rsem2, RSEM_PER_REP * (rep + 1)))
    srow = tiny.tile([1, NCORES], F32, tag="srow")
    sr = nc.vector.tensor_copy(
        srow, Y2[0:1, par, :, HC + 1 : HC + 2].rearrange("p r one -> p (r one)"))
    tile.add_dep_helper(sr.ins, x2w.ins, sync=False, reason="after AG2 arrival wait")
    negc2 = tiny.tile([1, 1], F32, tag="negc2")
    nc.vector.tensor_reduce(negc2, cmrow, axis=mybir.AxisListType.X,
                            op=mybir.AluOpType.max, negate=True)
    coef = tiny.tile([1, NCORES], F32, tag="coef")
    nc.scalar.activation(out=coef, in_=cmrow, func=mybir.ActivationFunctionType.Exp,
                         bias=negc2, scale=1.0)
    prod = tiny.tile([1, NCORES], F32, tag="prod")
    nc.vector.tensor_tensor(out=prod, in0=coef, in1=srow,
                            op=mybir.AluOpType.mult)
    D = tiny.tile([1, 1], F32, tag="D")
    nc.vector.reduce_sum(D, prod, axis=mybir.AxisListType.X)
    Sinv = tiny.tile([1, 1], F32, tag="Sinv")
    nc.vector.reciprocal(Sinv, D)
    csc = tiny.tile([1, NCORES], F32, tag="csc")
    nc.vector.tensor_scalar_mul(csc, coef, Sinv)
    # broadcast csc across partitions via ones-matmul
    ps_cb = psT.tile([128, NCORES], F32, tag="psT2")
    nc.tensor.matmul(ps_cb, ones_col[0:1, :].broadcast_to([1, 128]), csc,
                     start=True, stop=True)
    cscB = tiny.tile([128, NCORES], F32, tag="cscB")
    nc.vector.tensor_copy(cscB, ps_cb)
    # ep_col = sum_r cscB[:, r] * accep_col_r
    sc = tiny.tile([128, NCORES, HC], F32, tag="sc")
    for r in range(NCORES):
        nc.vector.tensor_scalar_mul(sc[:, r, :], Y2[:, par, r, 0:HC], cscB[:, r : r + 1])
    sa = tiny.tile([128, 4, HC], F32, tag="sa")
    sb = tiny.tile([128, 2, HC], F32, tag="sb")
    ep_col = tiny.tile([128, HC], F32, tag="ep_col")
    nc.vector.tensor_tensor(out=sa, in0=sc[:, 0:4, :], in1=sc[:, 4:8, :],
                            op=mybir.AluOpType.add)
    nc.vector.tensor_tensor(out=sb, in0=sa[:, 0:2, :], in1=sa[:, 2:4, :],
                            op=mybir.AluOpType.add)
    nc.vector.tensor_tensor(out=ep_col, in0=sb[:, 0, :], in1=sb[:, 1, :],
                            op=mybir.AluOpType.add)
    ceng.dma_start(out=out_ep.rearrange("a (c p) -> p (a c)", p=128), in_=ep_col)
    ep_colr = tiny.tile([128, HC], F32R, tag="ep_colr")
    nc.vector.tensor_copy(ep_colr, ep_col)

    # ---- v = W @ ep_.T  as a row: v_row = ep_ @ W.T  [1, H]
    ps_v = psA.tile([1, H], F32, tag="psA")
    for c in range(HC):
        for n0, nn in regions768:
            nc.tensor.matmul(
                ps_v[:, n0 : n0 + nn], ep_colr[:, c : c + 1], wt_ch[c][:, n0 : n0 + nn],
                start=(c == 0), stop=(c == HC - 1),
            )
    v_row = rows.tile([1, H], F32, tag="rows")
    nc.scalar.activation(out=v_row, in_=ps_v, func=mybir.ActivationFunctionType.Copy)
    vT = tiny.tile([128, HC], fdt, tag="vT")
    ps_t4 = psT.tile([128, HC], F32, tag="psT")
    for c in range(HC):
        nc.tensor.matmul(ps_t4[:, c : c + 1], v_row[:, c * 128 : (c + 1) * 128], ones11,
                         start=True, stop=True)
    nc.vector.tensor_copy(vT, ps_t4)

    if stage < 16:
        d2 = rows.tile([1, H + 2], F32, tag="row770")
        nc.vector.memset(d2, 0.0)
        nc.gpsimd.dma_start(out=out_st, in_=d2)
        return
    # ---- e2f local logits [1, NS] in float32r, transposed to [128, NT]
    # via K=1 matmuls (n = j*128 + p)
    x2 = psT.tile([128, NT], F32, tag="psX")
    _pend = None
    for b in range(NB):
        ps_e2f = psE.tile([1, 512], F32, tag="psE")
        for c in range(HC):
            nc.tensor.matmul(
                ps_e2f,
                vT[:, c : c + 1],
                factT_sb[:, c, b * 512 : (b + 1) * 512],
                start=(c == 0), stop=(c == HC - 1),
            )
        if _pend is not None:
            pb, prow = _pend
            for jj in range(4):
                j = pb * 4 + jj
                nc.tensor.matmul(x2[:, j : j + 1], prow[:, jj * 128 : (jj + 1) * 128],
                                 ones11, start=True, stop=True)
        e2f_row = rows.tile([1, 512], F32, tag="rows")
        nc.scalar.activation(out=e2f_row, in_=ps_e2f,
                             func=mybir.ActivationFunctionType.Copy)
        _pend = (b, e2f_row)
    pb, prow = _pend
    for jj in range(4):
        j = pb * 4 + jj
        nc.tensor.matmul(x2[:, j : j + 1], prow[:, jj * 128 : (jj + 1) * 128],
                         ones11, start=True, stop=True)

    # ---- e2f softmax partials (n = p*NT + j)
    m_p = tiny.tile([128, 1], F32, tag="m_p")
    nc.vector.reduce_max(m_p, x2, axis=mybir.AxisListType.X)
    ps_mr = psT.tile([1, 128], F32, tag="psT")
    nc.tensor.matmul(ps_mr, m_p, ident, start=True, stop=True)
    Cg = tiny.tile([1, 1], F32, tag="Cg")
    negCg = tiny.tile([1, 1], F32, tag="negCg")
    nc.vector.reduce_max(Cg, ps_mr, axis=mybir.AxisListType.X)
    nc.vector.tensor_reduce(negCg, ps_mr, axis=mybir.AxisListType.X,
                            op=mybir.AluOpType.max, negate=True)
    ps_cb2 = psT.tile([128, 1], F32, tag="psT2")
    nc.tensor.matmul(ps_cb2, ones_col[0:1, :].broadcast_to([1, 128]), negCg,
                     start=True, stop=True)
    negC = tiny.tile([128, 1], F32, tag="negC")
    nc.vector.tensor_copy(negC, ps_cb2)
    s_ps = tiny.tile([128, 1], F32, tag="s_ps")
    p2bf = tiny.tile([128, NT], BF16, tag="p2bf")
    nc.scalar.activation(
        out=p2bf, in_=x2, func=mybir.ActivationFunctionType.Exp,
        bias=negC, scale=1.0, accum_out=s_ps,
    )
    ps_S = psT.tile([1, 1], F32, tag="psT3")
    nc.tensor.matmul(ps_S, s_ps, ones_col, start=True, stop=True)

    if stage < 17:
        d2 = rows.tile([1, H + 2], F32, tag="row770")
        nc.vector.memset(d2, 0.0)
        nc.gpsimd.dma_start(out=out_st, in_=d2)
        return
    # ---- acc = sum_n p_n * fact[n, :]  (bf16 matmuls, fp32 accumulate)
    ps_acc = psA.tile([1, H], F32, tag="psA")
    for n0, nn in regions768:
        for j in range(NT):
            nc.tensor.matmul(
                ps_acc[:, n0 : n0 + nn], p2bf[:, j : j + 1], fnorm_sb[:, j, n0 : n0 + nn],
                start=(j == 0), stop=(j == NT - 1),
            )

    # ---- stats out: [C, S, acc]
    st_row = rows.tile([1, H + 2], F32, tag="row770")
    nc.vector.tensor_copy(st_row[:, 0:1], Cg)
    nc.vector.tensor_copy(st_row[:, 1:2], ps_S)
    nc.scalar.activation(out=st_row[:, 2 : H + 2], in_=ps_acc,
                         func=mybir.ActivationFunctionType.Copy)
    ceng.dma_start(out=out_st, in_=st_row)


def build(stage=999, reps=1, sim1=False, comm=None, dq=None, fdt=None, warm=None, ccq=None):
    fdt = {None: FDT, "f32r": "f32r", "bf16": "bf16"}[fdt]
    fdt = {"f32r": F32R, "bf16": BF16}[fdt]
    nc = bacc.Bacc("TRN2", target_bir_lowering=False, debug=False,
                   num_devices=1 if sim1 else NCORES)
    factT = nc.dram_tensor("factTb" if fdt == BF16 else "factT",
                           [H, NS], fdt, kind="ExternalInput").ap()
    factn = nc.dram_tensor("factn", [NS, H], BF16, kind="ExternalInput").ap()
    epT = nc.dram_tensor("epT", [H, MS], F32R, kind="ExternalInput").ap()
    epn = nc.dram_tensor("epn", [MS, H], F32R, kind="ExternalInput").ap()
    w = nc.dram_tensor("w", [H, H], F32R, kind="ExternalInput").ap()
    wt = nc.dram_tensor("wt", [H, H], F32R, kind="ExternalInput").ap()
    out_ep = nc.dram_tensor("out_ep", [1, H], F32, kind="ExternalOutput").ap()
    out_st = nc.dram_tensor("out_st", [1, H + 2], F32, kind="ExternalOutput").ap()
    from contextlib import ExitStack

    pools = {}
    with tile.TileContext(nc) as tc:
        with ExitStack() as ctx:
            for _ in range(reps):
                _body(nc, tc, ctx, factT, factn, epT, epn, w, wt, out_ep, out_st,
                      stage=stage, pools=pools, sim1=sim1, comm=comm, dq=dq, fdt=fdt,
                      warm=warm, ccq=ccq)
    # attach remote-arrival / drain waits after tile scheduling (the
    # scheduler's single-core sim cannot satisfy remote increments);
    # compile()'s generate_event_semaphores legalizes multi-wait insts
    for inst, sem, thr in pools.get("_waits", []):
        inst.wait_op(sem, thr, "sem-ge", check=False)
    nc.compile()
    return nc


def make_in_maps(fact, elements_p, weights):
    fact = np.asarray(fact, np.float32)
    elements_p = np.asarray(elements_p, np.float32)
    weights = np.asarray(weights, np.float32)
    wt = np.ascontiguousarray(weights.T)
    in_maps = []
    for i in range(NCORES):
        fs = fact[i * NS : (i + 1) * NS]
        es = elements_p[i * MS : (i + 1) * MS]
        fsT = np.ascontiguousarray(fs.T)
        in_maps.append({
            "factT": fsT,
            **({"factTb": fsT.astype(ml_dtypes.bfloat16)} if FDT == "bf16" else {}),
            "factn": fs.astype(ml_dtypes.bfloat16),
            "epT": np.ascontiguousarray(es.T),
            "epn": np.ascontiguousarray(es),
            "w": weights,
            "wt": wt,
        })
    return in_maps


def combine(results):
    st = np.stack([np.asarray(results[i]["out_st"][0], np.float64) for i in range(NCORES)])
    C = st[:, 0]
    S = st[:, 1]
    acc = st[:, 2:]
    Cg = C.max()
    coef = np.exp(C - Cg)
    fact_ = (coef[:, None] * acc).sum(0, keepdims=True) / (coef * S).sum()
    ep_ = np.asarray(results[0]["out_ep"], np.float32)
    return fact_.astype(np.float32), ep_


def kernel(fact, elements_p, weights, **run_kwargs):
    if "nc" not in _CACHE:
        _CACHE["nc"] = build()
    nc = _CACHE["nc"]
    in_maps = make_in_maps(fact, elements_p, weights)
    res = bass_utils.run_bass_kernel_spmd(
        nc, in_maps, core_ids=list(range(NCORES)), **run_kwargs
    )
    _CACHE["last_result"] = res
    return combine(res.results)


def build_sim1():
    return build(sim1=True)

